# revision 20
# baseline (speedup 1.0000x reference)
"""Trainium2 Bass kernel for GQA attention with KV cache — stream-paced pipeline.

Contract: kernel(**inputs) takes FULL unsharded inputs, returns FULL
[1, 128, 4096] output. Shards by KV head across 8 cores (core c owns kv head c
and q heads 4c..4c+3); wq/wk/wv column-parallel, wo row-parallel, fp16 partials
summed in f32 on host.

Design (HBM-stream-bound at ~12.6MB/core fp16; fp8 is numerically dead here —
softmax amplifies e4m3 score error to ~3e-2 > the 2e-2 gate):
- Single sync-queue input stream in exact consumption order:
  x, kcT, vct, wq0..wq3, wk, wv, wo (8 column chunks, last split in two).
  KV cache + x stream FIRST so the whole attention pipeline (scores, exp,
  transposes, AV, softmax normalization) closes while the wq/wk/wv weights
  stream; the 4MB wo stream is last and the po/ob/out-DMA chain chases it
  chunk-by-chunk, so the kernel tail after the last input byte is just one
  256-col chunk of compute + one small output DMA + the fixed Tile drain.
- Consts (identity, head-selector, causal mask) are generated ON-CHIP with
  GpSimd affine_select (no const DMAs, no mask bytes in the stream).
- q is projected PER HEAD with stationary=weight, moving=x so qT comes out of
  the PE directly; head h's scores/exp/transpose chain runs UNDER the stream.
- exps in 1024-wide blocks with activation(accum_out=) row sums; softmax
  normalization is deferred to the tiny yT tensor (recip broadcast via a PE
  outer product with an on-chip selector matrix).
- memset-sourced 512-col PE warmups bridge the DMA ramp so the HAM clock gate
  is at full rate when real matmuls start.
"""

import math
import sys

sys.path.insert(0, "/opt/trn_rl_repo")

import numpy as np

DIM = 4096
N_HEADS = 32
N_KV_HEADS = 8
HEAD_DIM = 128
N_REP = 4
MAX_SEQ = 4096
SEQ = 128
N_CORES = 8
NK = DIM // 128           # 32 contraction chunks
SB = 512                  # score block width over the old cache
EXP_SHIFT = 12.0
MASK_NEG = -30000.0
N_WARM = 16

_nc_cache = {}


def _build_nc(P):
    """Per-core Bass program (same on all 8 cores). P = input_pos."""
    import concourse.tile as tile
    from concourse import bacc, mybir
    from contextlib import ExitStack

    f32 = mybir.dt.float32
    DT = mybir.dt.float16
    AFT = mybir.ActivationFunctionType
    ALU = mybir.AluOpType

    assert P % SB == 0 and 0 < P <= MAX_SEQ - SEQ, f"unsupported input_pos {P}"
    NOLD = P // 128             # old-cache 128-col chunks (16)
    NCH = NOLD + 1              # +1 new block
    NB = P // SB                # 512-col transpose blocks per head (4)
    NB2 = P // 1024             # score/exp blocks per head (2)
    SCALE = 1.0 / math.sqrt(HEAD_DIM)

    nc = bacc.Bacc(None, target_bir_lowering=False)

    xT_d = nc.declare_dram_parameter("xTt", [128, NK * SEQ], DT, isOutput=False)
    wq_d = nc.declare_dram_parameter("wqt", [N_REP, 128, NK * HEAD_DIM], DT,
                                     isOutput=False)
    wk_d = nc.declare_dram_parameter("wkt", [128, NK * HEAD_DIM], DT, isOutput=False)
    wv_d = nc.declare_dram_parameter("wvt", [128, NK * HEAD_DIM], DT, isOutput=False)
    wo_d = nc.declare_dram_parameter("wot", [DIM // 512, 128, N_REP * 512], DT,
                                     isOutput=False)
    kcT_d = nc.declare_dram_parameter("kcT", [HEAD_DIM, P], DT, isOutput=False)
    vc_d = nc.declare_dram_parameter("vct", [128, NOLD * HEAD_DIM], DT, isOutput=False)
    out_d = nc.declare_dram_parameter("out", [SEQ, DIM], DT, isOutput=True)

    with tile.TileContext(nc) as tc, ExitStack() as ctx:
        const = ctx.enter_context(tc.tile_pool(name="const", bufs=1))
        persist = ctx.enter_context(tc.tile_pool(name="persist", bufs=1))
        wts = ctx.enter_context(tc.tile_pool(name="wts", bufs=1))
        attn_pool = ctx.enter_context(tc.tile_pool(name="attn", bufs=1))
        outp = ctx.enter_context(tc.tile_pool(name="outp", bufs=1))

        # ---- on-chip consts (memsets + GpSimd affine_select; no DMAs) ----
        shift_b = const.tile([128, 1], f32)
        nc.vector.memset(shift_b, -EXP_SHIFT)
        wf = const.tile([128, 512], f32)
        nc.vector.memset(wf, 1.0)
        wh4 = const.tile([128, 512], DT)
        nc.vector.tensor_copy(wh4, wf)
        zf = const.tile([SEQ, N_REP * 128], f32)
        nc.vector.memset(zf, 0.0)
        # ident[p, c] = (p == c)
        ident = const.tile([128, 128], DT)
        nc.gpsimd.affine_select(ident, wh4[:, 0:128], pattern=[[-1, 128]],
                                compare_op=ALU.is_equal, fill=0.0,
                                base=0, channel_multiplier=1)
        # sel4[k, h*128+a] = (k == h): stationary selector for recip broadcast
        sel4 = const.tile([N_REP, N_REP * 128], DT)
        nc.gpsimd.affine_select(sel4, wh4[0:N_REP, 0:N_REP * 128],
                                pattern=[[-1, N_REP], [0, 128]],
                                compare_op=ALU.is_equal, fill=0.0,
                                base=0, channel_multiplier=1)
        # mneg4[s, h, c] = 0 where c <= s else MASK_NEG (causal mask, f32)
        mneg4 = const.tile([SEQ, N_REP, 128], f32)
        nc.gpsimd.affine_select(mneg4.rearrange("p a b -> p (a b)"), zf,
                                pattern=[[0, N_REP], [-1, 128]],
                                compare_op=ALU.is_ge, fill=MASK_NEG,
                                base=0, channel_multiplier=1)

        # ---- input stream (sync queue, consumption order) ----
        # order: x, kcT, vct, wq0..3, wk, wv, wo0..6, wol(2 halves)
        xt = persist.tile([128, NK, SEQ], DT)
        nc.sync.dma_start(out=xt.rearrange("p a b -> p (a b)"), in_=xT_d[:, :])
        H = NK * HEAD_DIM
        wq_tiles = []
        KT = persist.tile([128, P + SEQ], DT)
        vsb = persist.tile([128, NOLD, HEAD_DIM], DT)
        for h in range(N_REP):
            t_ = wts.tile([128, NK, HEAD_DIM], DT, tag="wq", bufs=4, name=f"wq{h}")
            tv = t_.rearrange("p a b -> p (a b)")
            if h == 0:
                # wq0 streams right after x (ahead of the KV cache) so head 0's
                # projection/exp chain starts ~3us earlier and drains engine
                # queues before heads 1-3 arrive
                nc.sync.dma_start(out=tv[:, 0:H // 2], in_=wq_d[h, :, 0:H // 2])
                nc.sync.dma_start(out=tv[:, H // 2:], in_=wq_d[h, :, H // 2:])
                nc.sync.dma_start(out=KT[:, 0:P], in_=kcT_d[:, :])
                nc.sync.dma_start(out=vsb.rearrange("p a b -> p (a b)"),
                                  in_=vc_d[:, :])
            else:
                nc.sync.dma_start(out=tv, in_=wq_d[h, :, :])
            wq_tiles.append(t_)
        wkt = wts.tile([128, NK, HEAD_DIM], DT, tag="wk", bufs=1)
        kv_ = wkt.rearrange("p a b -> p (a b)")
        nc.sync.dma_start(out=kv_[:, 0:H // 2], in_=wk_d[:, 0:H // 2])
        nc.sync.dma_start(out=kv_[:, H // 2:], in_=wk_d[:, H // 2:])
        wvt = wts.tile([128, NK, HEAD_DIM], DT, tag="wv", bufs=1)
        vv_ = wvt.rearrange("p a b -> p (a b)")
        nc.sync.dma_start(out=vv_[:, 0:H // 2], in_=wv_d[:, 0:H // 2])
        nc.sync.dma_start(out=vv_[:, H // 2:], in_=wv_d[:, H // 2:])
        # last wo chunk is half-major on the host ([2, N_REP, 256]) and split
        # into two DMAs so the final out pipeline covers 256 cols, not 512
        wo_tiles = []
        NWO = DIM // 512
        for n in range(NWO - 1):
            t_ = wts.tile([128, N_REP, 512], DT, tag="wo", bufs=8, name=f"wo{n}")
            nc.sync.dma_start(out=t_.rearrange("p a b -> p (a b)"), in_=wo_d[n, :, :])
            wo_tiles.append(t_)
        wol = wts.tile([128, 2, N_REP, 256], DT, tag="wo", bufs=8, name="wol")
        wolv = wol.rearrange("p s a b -> p (s a b)")
        nc.sync.dma_start(out=wolv[:, 0:N_REP * 256], in_=wo_d[NWO - 1, :, 0:N_REP * 256])
        nc.sync.dma_start(out=wolv[:, N_REP * 256:], in_=wo_d[NWO - 1, :, N_REP * 256:])

        # ---- persistent SBUF state ----
        qT_sb = persist.tile([128, N_REP, SEQ], DT)
        vnew_sb = persist.tile([SEQ, HEAD_DIM], DT)
        attn_new = persist.tile([SEQ, N_REP, 128], DT)
        attnT = persist.tile([128, NCH, N_REP, 128], DT)
        yT_sb = persist.tile([128, N_REP, SEQ], DT)
        zp = persist.tile([128, N_REP, NB2 + 1], f32)
        zsum = persist.tile([128, N_REP, 1], f32)
        recip4 = persist.tile([128, N_REP], f32)
        recip4h = persist.tile([128, N_REP], DT)
        r4T_sb = persist.tile([N_REP, 128], DT)

        with tc.tile_pool(name="ps", bufs=1, space="PSUM") as ps:
            qT_ps = ps.tile([128, N_REP, SEQ], f32, tag="qT")
            yT_ps = ps.tile([128, N_REP * SEQ], f32, tag="yT")

            # PE warmup during the x/kcT/vct/wq0 stream: flips the HAM clock
            # gate to full rate before the q projection starts. Operands are
            # memset on-chip so the warmups have no DMA dependency.
            warm = ps.tile([128, 4, 128], f32, tag="tp", bufs=2)
            wv_ = warm.rearrange("p a b -> p (a b)")
            for _ in range(N_WARM):
                nc.tensor.matmul(wv_, wh4[:, 0:128], wh4[:],
                                 start=True, stop=True)

            attn_t = {}

            def q_head(h):
                for j in range(NK):
                    nc.tensor.matmul(qT_ps[:, h, :], wq_tiles[h][:, j, :],
                                     xt[:, j, :], start=(j == 0), stop=(j == NK - 1))
                nc.vector.tensor_copy(qT_sb[:, h, :], qT_ps[:, h, :])

            def s(h, b2):
                # 1024-wide exp blocks (2 score matmuls each), double-buffered
                # so block n+1's score matmuls run under block n's exp: the
                # chain is ACT-rate, not a per-block PSUM round-trip
                sc = ps.tile([SEQ, 1024], f32, tag="sc", bufs=2, name=f"sc{h}{b2}")
                for i in range(2):
                    off = b2 * 1024 + i * SB
                    nc.tensor.matmul(sc[:, i * SB:(i + 1) * SB], qT_sb[:, h, :],
                                     KT[:, off:off + SB], start=True, stop=True)
                a = attn_t[h][:, b2 * 1024:(b2 + 1) * 1024]
                nc.scalar.activation(a, sc[:], AFT.Exp, scale=SCALE,
                                     bias=shift_b[:],
                                     accum_out=zp[:, h, b2:b2 + 1])

            def t(h, b, copy_eng="v"):
                # transpose attn[h] chunks 4b..4b+3 -> attnT[:, 4b:4b+4, h, :]
                tp = ps.tile([128, 4, 128], f32, tag="tp", bufs=2, name=f"tp{h}{b}")
                for i in range(4):
                    c = 4 * b + i
                    nc.tensor.matmul(tp[:, i, :], attn_t[h][:, c * 128:(c + 1) * 128],
                                     ident[:], start=True, stop=True)
                dst = attnT[:, 4 * b:4 * b + 4, h, :]
                if copy_eng == "v":
                    nc.vector.tensor_copy(dst, tp[:])
                else:
                    nc.scalar.copy(dst, tp[:])

            def av(b):
                for i in range(4):
                    c = 4 * b + i
                    nc.tensor.matmul(yT_ps[:], vsb[:, c, :], attnT[:, c, :, :],
                                     start=(c == 0), stop=False)

            def newattn(h):
                attn_t[h] = attn_pool.tile([SEQ, P], DT, tag="attn", bufs=2,
                                           name=f"at{h}")

            # ---- pipelined emission (follows the stream order) ----
            # Per head: q(h) FIRST (its dep, the wq DMA, is the earliest),
            # then the previous head's transposes interleaved with the score
            # blocks — everything the PE FIFO reaches has its deps met, so
            # no head-of-line stalls.
            q_head(0)
            newattn(0)
            s(0, 0); s(0, 1)
            q_head(1)
            newattn(1)
            s(1, 0); s(1, 1)
            t(0, 0); t(0, 1); t(0, 2); t(0, 3)
            q_head(2)
            newattn(2)
            s(2, 0); s(2, 1)
            t(1, 0); t(1, 1); t(1, 2); t(1, 3)
            q_head(3)
            newattn(3)
            s(3, 0); s(3, 1)
            t(2, 0); t(2, 1); t(2, 2); t(2, 3)

            # k projection first: its dep (the wk stream) lands while the
            # head-3 exps are still running, so it must not queue behind the
            # head-3 transposes in the PE FIFO
            kv_ps = ps.tile([128, 4, 128], f32, tag="qT", name="kv")
            for j in range(NK):
                nc.tensor.matmul(kv_ps[:, 0, :], wkt[:, j, :], xt[:, j, :],
                                 start=(j == 0), stop=(j == NK - 1))
            nc.vector.tensor_copy(KT[:, P:P + SEQ], kv_ps[:, 0, :])

            # new-block scores for all heads (masked via mneg4 add pre-exp)
            nsc = ps.tile([128, N_REP, 128], f32, tag="sc", bufs=2, name="nsc")
            for h in range(N_REP):
                nc.tensor.matmul(nsc[:, h, :], qT_sb[:, h, :], KT[:, P:P + SEQ],
                                 start=True, stop=True)
            nscv = nsc.rearrange("p a b -> p (a b)")
            nc.vector.tensor_add(nscv, nscv, mneg4.rearrange("p a b -> p (a b)"))
            nc.scalar.activation(attn_new.rearrange("p a b -> p (a b)"), nscv,
                                 AFT.Exp, scale=SCALE, bias=shift_b[:])

            # v projection (wv is fully streamed by now)
            for j in range(NK):
                nc.tensor.matmul(kv_ps[:, 1, :], xt[:, j, :], wvt[:, j, :],
                                 start=(j == 0), stop=(j == NK - 1))
            nc.scalar.copy(vnew_sb[:], kv_ps[:, 1, :])

            # head-3 transposes: first two copies on DVE (its queue is short
            # until the z chain), last two on ACT right after the new exp
            t(3, 0, "v"); t(3, 1, "v"); t(3, 2, "s"); t(3, 3, "s")

            # AV chunks interleaved right behind the transpose copies
            av(0); av(1)

            # new-block attn transpose
            tpn = ps.tile([128, 4, 128], f32, tag="tp", bufs=2)
            for h in range(N_REP):
                nc.tensor.matmul(tpn[:, h, :], attn_new[:, h, :], ident[:],
                                 start=True, stop=True)
            nc.scalar.copy(attnT[:, NOLD, :, :], tpn[:])

            # softmax denominator chain on DVE (emitted before av(2/3) so
            # the recip broadcast closes under the AV tail)
            nc.vector.reduce_sum(zp[:, :, NB2:NB2 + 1], attn_new[:],
                                 axis=mybir.AxisListType.X)
            nc.vector.reduce_sum(zsum[:], zp[:], axis=mybir.AxisListType.X)
            nc.vector.reciprocal(recip4[:], zsum.rearrange("p a b -> p (a b)"))
            nc.vector.tensor_copy(recip4h[:], recip4[:])

            av(2)

            r4T_ps = ps.tile([N_REP, 128], f32, tag="tp", bufs=2, name="r4T")
            nc.tensor.matmul(r4T_ps[:], recip4h[:], ident[:], start=True, stop=True)
            nc.scalar.copy(r4T_sb[:], r4T_ps[:])
            rb_ps = ps.tile([128, N_REP, SEQ], f32, tag="sc", bufs=2, name="rb")
            for h in range(N_REP):
                nc.tensor.matmul(rb_ps[:, h, :], sel4[:, h * 128:(h + 1) * 128],
                                 r4T_sb[:], start=True, stop=True)

            av(3)

            nc.tensor.matmul(yT_ps[:], vnew_sb[:], attnT[:, NOLD, :, :],
                             start=False, stop=True)
            # finalize: copy raw yT to SBUF, then scale by rb straight from
            # PSUM (one PSUM operand is legal)
            nc.vector.tensor_copy(yT_sb.rearrange("p h s -> p (h s)"), yT_ps[:])
            nc.vector.tensor_mul(yT_sb.rearrange("p h s -> p (h s)"),
                                 yT_sb.rearrange("p h s -> p (h s)"),
                                 rb_ps.rearrange("p a b -> p (a b)"))

            # ---- wo (row-parallel partial), chasing the wo stream ----
            # ob copies on DVE, out-DMA issue on the otherwise-idle ACT queue
            # ob copies alternate DVE/ACT so consecutive po chunks' copies
            # overlap (the tp-slot rotation waits on the copy two chunks back)
            for n in range(NWO - 1):
                po = ps.tile([SEQ, SB], f32, tag="tp", bufs=2, name=f"po{n}")
                for h in range(N_REP):
                    nc.tensor.matmul(po[:], yT_sb[:, h, :], wo_tiles[n][:, h, :],
                                     start=(h == 0), stop=(h == N_REP - 1))
                ob = outp.tile([SEQ, 512], DT, tag="ob", bufs=3, name=f"ob{n}")
                if n % 2 == 0:
                    nc.vector.tensor_copy(ob[:], po[:])
                else:
                    nc.scalar.copy(ob[:], po[:])
                nc.sync.dma_start(out=out_d[:, n * 512:(n + 1) * 512], in_=ob[:])
            for half in range(2):
                po = ps.tile([SEQ, SB], f32, tag="tp", bufs=2, name=f"pol{half}")
                for h in range(N_REP):
                    nc.tensor.matmul(po[:, 0:256], yT_sb[:, h, :],
                                     wol[:, half, h, :],
                                     start=(h == 0), stop=(h == N_REP - 1))
                ob = outp.tile([SEQ, 512], DT, tag="ob", bufs=3, name=f"obl{half}")
                if half == 0:
                    nc.vector.tensor_copy(ob[:, 0:256], po[:, 0:256])
                else:
                    nc.scalar.copy(ob[:, 0:256], po[:, 0:256])
                off = (NWO - 1) * 512 + half * 256
                nc.sync.dma_start(out=out_d[:, off:off + 256], in_=ob[:, 0:256])

    nc.finalize()
    return nc


def _get_nc(P):
    if P not in _nc_cache:
        _nc_cache[P] = _build_nc(P)
    return _nc_cache[P]


def prep_in_maps(x, input_pos, k_cache, v_cache, wq, wk, wv, wo):
    P = int(input_pos)
    NOLD = P // 128
    x2 = np.asarray(x, dtype=np.float32).reshape(SEQ, DIM)
    k_cache = np.asarray(k_cache, dtype=np.float32)
    v_cache = np.asarray(v_cache, dtype=np.float32)
    wq = np.asarray(wq, dtype=np.float32)
    wk = np.asarray(wk, dtype=np.float32)
    wv = np.asarray(wv, dtype=np.float32)
    wo = np.asarray(wo, dtype=np.float32)

    xT = x2.T.astype(np.float16)                              # [DIM, SEQ]
    xTt = np.ascontiguousarray(
        xT.reshape(NK, 128, SEQ).transpose(1, 0, 2).reshape(128, NK * SEQ))

    def chunkT(wT):  # [DIM, C] -> [128, NK*C]: out[p, j*C+c] = wT[j*128+p, c]
        C = wT.shape[1]
        return np.ascontiguousarray(
            wT.reshape(NK, 128, C).transpose(1, 0, 2).reshape(128, NK * C))

    in_maps = []
    for c in range(N_CORES):
        wq_c = wq[c * 512:(c + 1) * 512]                      # [512, DIM]
        wqt = np.stack([chunkT(wq_c[h * 128:(h + 1) * 128].T.astype(np.float16))
                        for h in range(N_REP)])               # [4, 128, NK*128]
        wkt = chunkT(wk[c * 128:(c + 1) * 128].T.astype(np.float16))
        wvt = chunkT(wv[c * 128:(c + 1) * 128].T.astype(np.float16))
        woT = wo[:, c * 512:(c + 1) * 512].T.astype(np.float16)  # [512, DIM]
        # wot[n][p, h*512+col] = woT[h*128+p, n*512+col]
        wot = np.ascontiguousarray(
            woT.reshape(N_REP, 128, DIM // 512, 512)
            .transpose(2, 1, 0, 3).reshape(DIM // 512, 128, N_REP * 512))
        # last chunk half-major: wot[-1][p, s*1024 + h*256 + col]
        wol = wot[-1].reshape(128, N_REP, 2, 256).transpose(0, 2, 1, 3)
        wot = np.concatenate(
            [wot[:-1], wol.reshape(1, 128, N_REP * 512)], axis=0)
        m = {
            "xTt": xTt,
            "wqt": wqt,
            "wkt": wkt,
            "wvt": wvt,
            "wot": wot,
            "kcT": np.ascontiguousarray(k_cache[0, c, :P].T.astype(np.float16)),
            "vct": np.ascontiguousarray(
                v_cache[0, c, :P].astype(np.float16)
                .reshape(NOLD, 128, HEAD_DIM).transpose(1, 0, 2).reshape(128, -1)),
        }
        in_maps.append(m)
    return P, in_maps


def kernel(x, input_pos, k_cache, v_cache, wq, wk, wv, wo):
    from concourse.bass_utils import run_bass_kernel_spmd

    P, in_maps = prep_in_maps(x, input_pos, k_cache, v_cache, wq, wk, wv, wo)
    nc = _get_nc(P)
    res = run_bass_kernel_spmd(nc, in_maps, core_ids=list(range(N_CORES)))
    out = np.zeros((SEQ, DIM), dtype=np.float32)
    for r in res.results:
        out += r["out"].astype(np.float32)
    return out.reshape(1, SEQ, DIM)


if __name__ == "__main__":
    rng = np.random.default_rng(0)
    ins = {
        "x": rng.standard_normal((1, SEQ, DIM), dtype=np.float32),
        "input_pos": 2048,
        "k_cache": rng.standard_normal((1, N_KV_HEADS, MAX_SEQ, HEAD_DIM),
                                       dtype=np.float32),
        "v_cache": rng.standard_normal((1, N_KV_HEADS, MAX_SEQ, HEAD_DIM),
                                       dtype=np.float32),
        "wq": rng.standard_normal((N_HEADS * HEAD_DIM, DIM), dtype=np.float32) * 0.02,
        "wk": rng.standard_normal((N_KV_HEADS * HEAD_DIM, DIM),
                                  dtype=np.float32) * 0.02,
        "wv": rng.standard_normal((N_KV_HEADS * HEAD_DIM, DIM),
                                  dtype=np.float32) * 0.02,
        "wo": rng.standard_normal((DIM, N_HEADS * HEAD_DIM), dtype=np.float32) * 0.02,
    }
    out = kernel(**ins)
    print("out", out.shape, out.dtype, float(np.abs(out).max()))


# revision 21
# speedup vs baseline: 1.0111x; 1.0111x over previous
"""Trainium2 Bass kernel for GQA attention with KV cache — stream-paced pipeline.

Contract: kernel(**inputs) takes FULL unsharded inputs, returns FULL
[1, 128, 4096] output. Shards by KV head across 8 cores (core c owns kv head c
and q heads 4c..4c+3); wq/wk/wv column-parallel, wo row-parallel, fp16 partials
summed in f32 on host.

Design (HBM-stream-bound at ~12.6MB/core fp16; fp8 is numerically dead here —
softmax amplifies e4m3 score error to ~3e-2 > the 2e-2 gate):
- Single sync-queue input stream in exact consumption order:
  x, kcT, vct, wq0..wq3, wk, wv, wo (8 column chunks, last split in two).
  KV cache + x stream FIRST so the whole attention pipeline (scores, exp,
  transposes, AV, softmax normalization) closes while the wq/wk/wv weights
  stream; the 4MB wo stream is last and the po/ob/out-DMA chain chases it
  chunk-by-chunk, so the kernel tail after the last input byte is just one
  256-col chunk of compute + one small output DMA + the fixed Tile drain.
- Consts (identity, head-selector, causal mask) are generated ON-CHIP with
  GpSimd affine_select (no const DMAs, no mask bytes in the stream).
- q is projected PER HEAD with stationary=weight, moving=x so qT comes out of
  the PE directly; head h's scores/exp/transpose chain runs UNDER the stream.
- exps in 1024-wide blocks with activation(accum_out=) row sums; softmax
  normalization is deferred to the tiny yT tensor (recip broadcast via a PE
  outer product with an on-chip selector matrix).
- memset-sourced 512-col PE warmups bridge the DMA ramp so the HAM clock gate
  is at full rate when real matmuls start.
"""

import math
import sys

sys.path.insert(0, "/opt/trn_rl_repo")

import numpy as np

DIM = 4096
N_HEADS = 32
N_KV_HEADS = 8
HEAD_DIM = 128
N_REP = 4
MAX_SEQ = 4096
SEQ = 128
N_CORES = 8
NK = DIM // 128           # 32 contraction chunks
SB = 512                  # score block width over the old cache
EXP_SHIFT = 12.0
MASK_NEG = -30000.0
N_WARM = 24

_nc_cache = {}


def _build_nc(P):
    """Per-core Bass program (same on all 8 cores). P = input_pos."""
    import concourse.tile as tile
    from concourse import bacc, mybir
    from contextlib import ExitStack

    f32 = mybir.dt.float32
    DT = mybir.dt.float16
    AFT = mybir.ActivationFunctionType
    ALU = mybir.AluOpType

    assert P % SB == 0 and 0 < P <= MAX_SEQ - SEQ, f"unsupported input_pos {P}"
    NOLD = P // 128             # old-cache 128-col chunks (16)
    NCH = NOLD + 1              # +1 new block
    NB = P // SB                # 512-col transpose blocks per head (4)
    NB2 = P // 1024             # score/exp blocks per head (2)
    SCALE = 1.0 / math.sqrt(HEAD_DIM)

    nc = bacc.Bacc(None, target_bir_lowering=False)

    xT_d = nc.declare_dram_parameter("xTt", [128, NK * SEQ], DT, isOutput=False)
    wq_d = nc.declare_dram_parameter("wqt", [N_REP, 128, NK * HEAD_DIM], DT,
                                     isOutput=False)
    wk_d = nc.declare_dram_parameter("wkt", [128, NK * HEAD_DIM], DT, isOutput=False)
    wv_d = nc.declare_dram_parameter("wvt", [128, NK * HEAD_DIM], DT, isOutput=False)
    wo_d = nc.declare_dram_parameter("wot", [DIM // 512, 128, N_REP * 512], DT,
                                     isOutput=False)
    kcT_d = nc.declare_dram_parameter("kcT", [HEAD_DIM, P], DT, isOutput=False)
    vc_d = nc.declare_dram_parameter("vct", [128, NOLD * HEAD_DIM], DT, isOutput=False)
    out_d = nc.declare_dram_parameter("out", [SEQ, DIM], DT, isOutput=True)

    with tile.TileContext(nc) as tc, ExitStack() as ctx:
        const = ctx.enter_context(tc.tile_pool(name="const", bufs=1))
        persist = ctx.enter_context(tc.tile_pool(name="persist", bufs=1))
        wts = ctx.enter_context(tc.tile_pool(name="wts", bufs=1))
        attn_pool = ctx.enter_context(tc.tile_pool(name="attn", bufs=1))
        outp = ctx.enter_context(tc.tile_pool(name="outp", bufs=1))

        # ---- on-chip consts (memsets + GpSimd affine_select; no DMAs) ----
        shift_b = const.tile([128, 1], f32)
        nc.vector.memset(shift_b, -EXP_SHIFT)
        wf = const.tile([128, 512], f32)
        nc.vector.memset(wf, 1.0)
        wh4 = const.tile([128, 512], DT)
        nc.vector.tensor_copy(wh4, wf)
        zf = const.tile([SEQ, N_REP * 128], f32)
        nc.vector.memset(zf, 0.0)
        # ident[p, c] = (p == c)
        ident = const.tile([128, 128], DT)
        nc.gpsimd.affine_select(ident, wh4[:, 0:128], pattern=[[-1, 128]],
                                compare_op=ALU.is_equal, fill=0.0,
                                base=0, channel_multiplier=1)
        # sel4[k, h*128+a] = (k == h): stationary selector for recip broadcast
        sel4 = const.tile([N_REP, N_REP * 128], DT)
        nc.gpsimd.affine_select(sel4, wh4[0:N_REP, 0:N_REP * 128],
                                pattern=[[-1, N_REP], [0, 128]],
                                compare_op=ALU.is_equal, fill=0.0,
                                base=0, channel_multiplier=1)
        # mneg4[s, h, c] = 0 where c <= s else MASK_NEG (causal mask, f32)
        mneg4 = const.tile([SEQ, N_REP, 128], f32)
        nc.gpsimd.affine_select(mneg4.rearrange("p a b -> p (a b)"), zf,
                                pattern=[[0, N_REP], [-1, 128]],
                                compare_op=ALU.is_ge, fill=MASK_NEG,
                                base=0, channel_multiplier=1)

        # ---- input stream (sync queue, consumption order) ----
        # order: x, kcT, vct, wq0..3, wk, wv, wo0..6, wol(2 halves)
        xt = persist.tile([128, NK, SEQ], DT)
        nc.sync.dma_start(out=xt.rearrange("p a b -> p (a b)"), in_=xT_d[:, :])
        KT = persist.tile([128, P + SEQ], DT)
        nc.sync.dma_start(out=KT[:, 0:P], in_=kcT_d[:, :])
        vsb = persist.tile([128, NOLD, HEAD_DIM], DT)
        nc.sync.dma_start(out=vsb.rearrange("p a b -> p (a b)"), in_=vc_d[:, :])
        H = NK * HEAD_DIM
        wq_tiles = []
        for h in range(N_REP):
            t_ = wts.tile([128, NK, HEAD_DIM], DT, tag="wq", bufs=4, name=f"wq{h}")
            tv = t_.rearrange("p a b -> p (a b)")
            if h == 0:
                nc.sync.dma_start(out=tv[:, 0:H // 2], in_=wq_d[h, :, 0:H // 2])
                nc.sync.dma_start(out=tv[:, H // 2:], in_=wq_d[h, :, H // 2:])
            else:
                nc.sync.dma_start(out=tv, in_=wq_d[h, :, :])
            wq_tiles.append(t_)
        wkt = wts.tile([128, NK, HEAD_DIM], DT, tag="wk", bufs=1)
        kv_ = wkt.rearrange("p a b -> p (a b)")
        nc.sync.dma_start(out=kv_[:, 0:H // 2], in_=wk_d[:, 0:H // 2])
        nc.sync.dma_start(out=kv_[:, H // 2:], in_=wk_d[:, H // 2:])
        wvt = wts.tile([128, NK, HEAD_DIM], DT, tag="wv", bufs=1)
        vv_ = wvt.rearrange("p a b -> p (a b)")
        nc.sync.dma_start(out=vv_[:, 0:H // 2], in_=wv_d[:, 0:H // 2])
        nc.sync.dma_start(out=vv_[:, H // 2:], in_=wv_d[:, H // 2:])
        # last wo chunk is half-major on the host ([2, N_REP, 256]) and split
        # into two DMAs so the final out pipeline covers 256 cols, not 512
        wo_tiles = []
        NWO = DIM // 512
        for n in range(NWO - 1):
            t_ = wts.tile([128, N_REP, 512], DT, tag="wo", bufs=8, name=f"wo{n}")
            nc.sync.dma_start(out=t_.rearrange("p a b -> p (a b)"), in_=wo_d[n, :, :])
            wo_tiles.append(t_)
        wol = wts.tile([128, 2, N_REP, 256], DT, tag="wo", bufs=8, name="wol")
        wolv = wol.rearrange("p s a b -> p (s a b)")
        nc.sync.dma_start(out=wolv[:, 0:N_REP * 256], in_=wo_d[NWO - 1, :, 0:N_REP * 256])
        nc.sync.dma_start(out=wolv[:, N_REP * 256:], in_=wo_d[NWO - 1, :, N_REP * 256:])

        # ---- persistent SBUF state ----
        qT_sb = persist.tile([128, N_REP, SEQ], DT)
        vnew_sb = persist.tile([SEQ, HEAD_DIM], DT)
        attn_new = persist.tile([SEQ, N_REP, 128], DT)
        attnT = persist.tile([128, NCH, N_REP, 128], DT)
        yT_sb = persist.tile([128, N_REP, SEQ], DT)
        zp = persist.tile([128, N_REP, NB2 + 1], f32)
        zsum = persist.tile([128, N_REP, 1], f32)
        recip4 = persist.tile([128, N_REP], f32)
        recip4h = persist.tile([128, N_REP], DT)
        r4T_sb = persist.tile([N_REP, 128], DT)

        with tc.tile_pool(name="ps", bufs=1, space="PSUM") as ps:
            qT_ps = ps.tile([128, N_REP, SEQ], f32, tag="qT")
            yT_ps = ps.tile([128, N_REP * SEQ], f32, tag="yT")

            # PE warmup during the x/kcT/vct/wq0 stream: flips the HAM clock
            # gate to full rate before the q projection starts. Operands are
            # memset on-chip so the warmups have no DMA dependency.
            warm = ps.tile([128, 4, 128], f32, tag="tp", bufs=2)
            wv_ = warm.rearrange("p a b -> p (a b)")
            for _ in range(N_WARM):
                nc.tensor.matmul(wv_, wh4[:, 0:128], wh4[:],
                                 start=True, stop=True)

            attn_t = {}

            def q_head(h):
                for j in range(NK):
                    nc.tensor.matmul(qT_ps[:, h, :], wq_tiles[h][:, j, :],
                                     xt[:, j, :], start=(j == 0), stop=(j == NK - 1))
                nc.vector.tensor_copy(qT_sb[:, h, :], qT_ps[:, h, :])

            def s(h, b2):
                # 1024-wide exp blocks (2 score matmuls each), double-buffered
                # so block n+1's score matmuls run under block n's exp: the
                # chain is ACT-rate, not a per-block PSUM round-trip
                sc = ps.tile([SEQ, 1024], f32, tag="sc", bufs=2, name=f"sc{h}{b2}")
                for i in range(2):
                    off = b2 * 1024 + i * SB
                    nc.tensor.matmul(sc[:, i * SB:(i + 1) * SB], qT_sb[:, h, :],
                                     KT[:, off:off + SB], start=True, stop=True)
                a = attn_t[h][:, b2 * 1024:(b2 + 1) * 1024]
                nc.scalar.activation(a, sc[:], AFT.Exp, scale=SCALE,
                                     bias=shift_b[:],
                                     accum_out=zp[:, h, b2:b2 + 1])

            def t(h, b, copy_eng="v"):
                # transpose attn[h] chunks 4b..4b+3 -> attnT[:, 4b:4b+4, h, :]
                tp = ps.tile([128, 4, 128], f32, tag="tp", bufs=2, name=f"tp{h}{b}")
                for i in range(4):
                    c = 4 * b + i
                    nc.tensor.matmul(tp[:, i, :], attn_t[h][:, c * 128:(c + 1) * 128],
                                     ident[:], start=True, stop=True)
                dst = attnT[:, 4 * b:4 * b + 4, h, :]
                if copy_eng == "v":
                    nc.vector.tensor_copy(dst, tp[:])
                else:
                    nc.scalar.copy(dst, tp[:])

            def av(b):
                for i in range(4):
                    c = 4 * b + i
                    nc.tensor.matmul(yT_ps[:], vsb[:, c, :], attnT[:, c, :, :],
                                     start=(c == 0), stop=False)

            def newattn(h):
                attn_t[h] = attn_pool.tile([SEQ, P], DT, tag="attn", bufs=2,
                                           name=f"at{h}")

            # ---- pipelined emission (follows the stream order) ----
            # Per head: q(h) FIRST (its dep, the wq DMA, is the earliest),
            # then the previous head's transposes interleaved with the score
            # blocks — everything the PE FIFO reaches has its deps met, so
            # no head-of-line stalls.
            q_head(0)
            newattn(0)
            s(0, 0); s(0, 1)
            q_head(1)
            newattn(1)
            s(1, 0); s(1, 1)
            t(0, 0); t(0, 1); t(0, 2); t(0, 3)
            q_head(2)
            newattn(2)
            s(2, 0); s(2, 1)
            t(1, 0); t(1, 1); t(1, 2); t(1, 3)
            q_head(3)
            newattn(3)
            s(3, 0); s(3, 1)
            t(2, 0); t(2, 1); t(2, 2); t(2, 3)

            # k projection first: its dep (the wk stream) lands while the
            # head-3 exps are still running, so it must not queue behind the
            # head-3 transposes in the PE FIFO
            kv_ps = ps.tile([128, 4, 128], f32, tag="qT", name="kv")
            for j in range(NK):
                nc.tensor.matmul(kv_ps[:, 0, :], wkt[:, j, :], xt[:, j, :],
                                 start=(j == 0), stop=(j == NK - 1))
            nc.vector.tensor_copy(KT[:, P:P + SEQ], kv_ps[:, 0, :])

            # new-block scores for all heads (masked via mneg4 add pre-exp)
            nsc = ps.tile([128, N_REP, 128], f32, tag="sc", bufs=2, name="nsc")
            for h in range(N_REP):
                nc.tensor.matmul(nsc[:, h, :], qT_sb[:, h, :], KT[:, P:P + SEQ],
                                 start=True, stop=True)
            nscv = nsc.rearrange("p a b -> p (a b)")
            nc.vector.tensor_add(nscv, nscv, mneg4.rearrange("p a b -> p (a b)"))
            nc.scalar.activation(attn_new.rearrange("p a b -> p (a b)"), nscv,
                                 AFT.Exp, scale=SCALE, bias=shift_b[:])

            # v projection (wv is fully streamed by now)
            for j in range(NK):
                nc.tensor.matmul(kv_ps[:, 1, :], xt[:, j, :], wvt[:, j, :],
                                 start=(j == 0), stop=(j == NK - 1))
            nc.scalar.copy(vnew_sb[:], kv_ps[:, 1, :])

            # head-3 transposes: first two copies on DVE (its queue is short
            # until the z chain), last two on ACT right after the new exp
            t(3, 0, "v"); t(3, 1, "v"); t(3, 2, "s"); t(3, 3, "s")

            # AV chunks interleaved right behind the transpose copies
            av(0); av(1)

            # new-block attn transpose
            tpn = ps.tile([128, 4, 128], f32, tag="tp", bufs=2)
            for h in range(N_REP):
                nc.tensor.matmul(tpn[:, h, :], attn_new[:, h, :], ident[:],
                                 start=True, stop=True)
            nc.scalar.copy(attnT[:, NOLD, :, :], tpn[:])

            av(2); av(3)

            # softmax denominator chain on DVE
            nc.vector.reduce_sum(zp[:, :, NB2:NB2 + 1], attn_new[:],
                                 axis=mybir.AxisListType.X)
            nc.vector.reduce_sum(zsum[:], zp[:], axis=mybir.AxisListType.X)
            nc.vector.reciprocal(recip4[:], zsum.rearrange("p a b -> p (a b)"))
            nc.vector.tensor_copy(recip4h[:], recip4[:])

            r4T_ps = ps.tile([N_REP, 128], f32, tag="tp", bufs=2, name="r4T")
            nc.tensor.matmul(r4T_ps[:], recip4h[:], ident[:], start=True, stop=True)
            nc.scalar.copy(r4T_sb[:], r4T_ps[:])
            rb_ps = ps.tile([128, N_REP, SEQ], f32, tag="sc", bufs=2, name="rb")
            for h in range(N_REP):
                nc.tensor.matmul(rb_ps[:, h, :], sel4[:, h * 128:(h + 1) * 128],
                                 r4T_sb[:], start=True, stop=True)

            nc.tensor.matmul(yT_ps[:], vnew_sb[:], attnT[:, NOLD, :, :],
                             start=False, stop=True)
            # finalize: copy raw yT to SBUF, then scale by rb straight from
            # PSUM (one PSUM operand is legal)
            nc.vector.tensor_copy(yT_sb.rearrange("p h s -> p (h s)"), yT_ps[:])
            nc.vector.tensor_mul(yT_sb.rearrange("p h s -> p (h s)"),
                                 yT_sb.rearrange("p h s -> p (h s)"),
                                 rb_ps.rearrange("p a b -> p (a b)"))

            # ---- wo (row-parallel partial), chasing the wo stream ----
            # ob copies on DVE, out-DMA issue on the otherwise-idle ACT queue
            # ob copies alternate DVE/ACT so consecutive po chunks' copies
            # overlap (the tp-slot rotation waits on the copy two chunks back)
            for n in range(NWO - 1):
                po = ps.tile([SEQ, SB], f32, tag="tp", bufs=2, name=f"po{n}")
                for h in range(N_REP):
                    nc.tensor.matmul(po[:], yT_sb[:, h, :], wo_tiles[n][:, h, :],
                                     start=(h == 0), stop=(h == N_REP - 1))
                ob = outp.tile([SEQ, 512], DT, tag="ob", bufs=3, name=f"ob{n}")
                if n % 2 == 0:
                    nc.vector.tensor_copy(ob[:], po[:])
                else:
                    nc.scalar.copy(ob[:], po[:])
                nc.sync.dma_start(out=out_d[:, n * 512:(n + 1) * 512], in_=ob[:])
            for half in range(2):
                po = ps.tile([SEQ, SB], f32, tag="tp", bufs=2, name=f"pol{half}")
                for h in range(N_REP):
                    nc.tensor.matmul(po[:, 0:256], yT_sb[:, h, :],
                                     wol[:, half, h, :],
                                     start=(h == 0), stop=(h == N_REP - 1))
                ob = outp.tile([SEQ, 512], DT, tag="ob", bufs=3, name=f"obl{half}")
                if half == 0:
                    nc.vector.tensor_copy(ob[:, 0:256], po[:, 0:256])
                else:
                    nc.scalar.copy(ob[:, 0:256], po[:, 0:256])
                off = (NWO - 1) * 512 + half * 256
                nc.sync.dma_start(out=out_d[:, off:off + 256], in_=ob[:, 0:256])

    nc.finalize()
    return nc


def _get_nc(P):
    if P not in _nc_cache:
        _nc_cache[P] = _build_nc(P)
    return _nc_cache[P]


def prep_in_maps(x, input_pos, k_cache, v_cache, wq, wk, wv, wo):
    P = int(input_pos)
    NOLD = P // 128
    x2 = np.asarray(x, dtype=np.float32).reshape(SEQ, DIM)
    k_cache = np.asarray(k_cache, dtype=np.float32)
    v_cache = np.asarray(v_cache, dtype=np.float32)
    wq = np.asarray(wq, dtype=np.float32)
    wk = np.asarray(wk, dtype=np.float32)
    wv = np.asarray(wv, dtype=np.float32)
    wo = np.asarray(wo, dtype=np.float32)

    xT = x2.T.astype(np.float16)                              # [DIM, SEQ]
    xTt = np.ascontiguousarray(
        xT.reshape(NK, 128, SEQ).transpose(1, 0, 2).reshape(128, NK * SEQ))

    def chunkT(wT):  # [DIM, C] -> [128, NK*C]: out[p, j*C+c] = wT[j*128+p, c]
        C = wT.shape[1]
        return np.ascontiguousarray(
            wT.reshape(NK, 128, C).transpose(1, 0, 2).reshape(128, NK * C))

    in_maps = []
    for c in range(N_CORES):
        wq_c = wq[c * 512:(c + 1) * 512]                      # [512, DIM]
        wqt = np.stack([chunkT(wq_c[h * 128:(h + 1) * 128].T.astype(np.float16))
                        for h in range(N_REP)])               # [4, 128, NK*128]
        wkt = chunkT(wk[c * 128:(c + 1) * 128].T.astype(np.float16))
        wvt = chunkT(wv[c * 128:(c + 1) * 128].T.astype(np.float16))
        woT = wo[:, c * 512:(c + 1) * 512].T.astype(np.float16)  # [512, DIM]
        # wot[n][p, h*512+col] = woT[h*128+p, n*512+col]
        wot = np.ascontiguousarray(
            woT.reshape(N_REP, 128, DIM // 512, 512)
            .transpose(2, 1, 0, 3).reshape(DIM // 512, 128, N_REP * 512))
        # last chunk half-major: wot[-1][p, s*1024 + h*256 + col]
        wol = wot[-1].reshape(128, N_REP, 2, 256).transpose(0, 2, 1, 3)
        wot = np.concatenate(
            [wot[:-1], wol.reshape(1, 128, N_REP * 512)], axis=0)
        m = {
            "xTt": xTt,
            "wqt": wqt,
            "wkt": wkt,
            "wvt": wvt,
            "wot": wot,
            "kcT": np.ascontiguousarray(k_cache[0, c, :P].T.astype(np.float16)),
            "vct": np.ascontiguousarray(
                v_cache[0, c, :P].astype(np.float16)
                .reshape(NOLD, 128, HEAD_DIM).transpose(1, 0, 2).reshape(128, -1)),
        }
        in_maps.append(m)
    return P, in_maps


def kernel(x, input_pos, k_cache, v_cache, wq, wk, wv, wo):
    from concourse.bass_utils import run_bass_kernel_spmd

    P, in_maps = prep_in_maps(x, input_pos, k_cache, v_cache, wq, wk, wv, wo)
    nc = _get_nc(P)
    res = run_bass_kernel_spmd(nc, in_maps, core_ids=list(range(N_CORES)))
    out = np.zeros((SEQ, DIM), dtype=np.float32)
    for r in res.results:
        out += r["out"].astype(np.float32)
    return out.reshape(1, SEQ, DIM)


if __name__ == "__main__":
    rng = np.random.default_rng(0)
    ins = {
        "x": rng.standard_normal((1, SEQ, DIM), dtype=np.float32),
        "input_pos": 2048,
        "k_cache": rng.standard_normal((1, N_KV_HEADS, MAX_SEQ, HEAD_DIM),
                                       dtype=np.float32),
        "v_cache": rng.standard_normal((1, N_KV_HEADS, MAX_SEQ, HEAD_DIM),
                                       dtype=np.float32),
        "wq": rng.standard_normal((N_HEADS * HEAD_DIM, DIM), dtype=np.float32) * 0.02,
        "wk": rng.standard_normal((N_KV_HEADS * HEAD_DIM, DIM),
                                  dtype=np.float32) * 0.02,
        "wv": rng.standard_normal((N_KV_HEADS * HEAD_DIM, DIM),
                                  dtype=np.float32) * 0.02,
        "wo": rng.standard_normal((DIM, N_HEADS * HEAD_DIM), dtype=np.float32) * 0.02,
    }
    out = kernel(**ins)
    print("out", out.shape, out.dtype, float(np.abs(out).max()))


# revision 22
# speedup vs baseline: 1.0269x; 1.0157x over previous
"""Trainium2 Bass kernel for GQA attention with KV cache — stream-paced pipeline.

Contract: kernel(**inputs) takes FULL unsharded inputs, returns FULL
[1, 128, 4096] output. Shards by KV head across 8 cores (core c owns kv head c
and q heads 4c..4c+3); wq/wk/wv column-parallel, wo row-parallel, fp16 partials
summed in f32 on host.

Design (HBM-stream-bound at ~12.6MB/core fp16; fp8 is numerically dead here —
softmax amplifies e4m3 score error to ~3e-2 > the 2e-2 gate):
- Single sync-queue input stream in exact consumption order:
  x, kcT, vct, wq0..wq3, wk, wv, wo (8 column chunks, last split in two).
  KV cache + x stream FIRST so the whole attention pipeline (scores, exp,
  transposes, AV, softmax normalization) closes while the wq/wk/wv weights
  stream; the 4MB wo stream is last and the po/ob/out-DMA chain chases it
  chunk-by-chunk, so the kernel tail after the last input byte is just one
  256-col chunk of compute + one small output DMA + the fixed Tile drain.
- Consts (identity, head-selector, causal mask) are generated ON-CHIP with
  GpSimd affine_select (no const DMAs, no mask bytes in the stream).
- q is projected PER HEAD with stationary=weight, moving=x so qT comes out of
  the PE directly; head h's scores/exp/transpose chain runs UNDER the stream.
- exps in 1024-wide blocks with activation(accum_out=) row sums; softmax
  normalization is deferred to the tiny yT tensor (recip broadcast via a PE
  outer product with an on-chip selector matrix).
- memset-sourced 512-col PE warmups bridge the DMA ramp so the HAM clock gate
  is at full rate when real matmuls start.
"""

import math
import sys

sys.path.insert(0, "/opt/trn_rl_repo")

import numpy as np

DIM = 4096
N_HEADS = 32
N_KV_HEADS = 8
HEAD_DIM = 128
N_REP = 4
MAX_SEQ = 4096
SEQ = 128
N_CORES = 8
NK = DIM // 128           # 32 contraction chunks
SB = 512                  # score block width over the old cache
EXP_SHIFT = 12.0
MASK_NEG = -30000.0
N_WARM = 24

_nc_cache = {}


def _build_nc(P):
    """Per-core Bass program (same on all 8 cores). P = input_pos."""
    import concourse.tile as tile
    from concourse import bacc, mybir
    from contextlib import ExitStack

    f32 = mybir.dt.float32
    DT = mybir.dt.float16
    AFT = mybir.ActivationFunctionType
    ALU = mybir.AluOpType

    assert P % SB == 0 and 0 < P <= MAX_SEQ - SEQ, f"unsupported input_pos {P}"
    NOLD = P // 128             # old-cache 128-col chunks (16)
    NCH = NOLD + 1              # +1 new block
    NB = P // SB                # 512-col transpose blocks per head (4)
    NB2 = P // 1024             # score/exp blocks per head (2)
    SCALE = 1.0 / math.sqrt(HEAD_DIM)

    nc = bacc.Bacc(None, target_bir_lowering=False)

    xT_d = nc.declare_dram_parameter("xTt", [128, NK * SEQ], DT, isOutput=False)
    wq_d = nc.declare_dram_parameter("wqt", [N_REP, 128, NK * HEAD_DIM], DT,
                                     isOutput=False)
    wk_d = nc.declare_dram_parameter("wkt", [128, NK * HEAD_DIM], DT, isOutput=False)
    wv_d = nc.declare_dram_parameter("wvt", [128, NK * HEAD_DIM], DT, isOutput=False)
    wo_d = nc.declare_dram_parameter("wot", [DIM // 512, 128, N_REP * 512], DT,
                                     isOutput=False)
    kcT_d = nc.declare_dram_parameter("kcT", [HEAD_DIM, P], DT, isOutput=False)
    vc_d = nc.declare_dram_parameter("vct", [128, NOLD * HEAD_DIM], DT, isOutput=False)
    out_d = nc.declare_dram_parameter("out", [SEQ, DIM], DT, isOutput=True)

    with tile.TileContext(nc) as tc, ExitStack() as ctx:
        const = ctx.enter_context(tc.tile_pool(name="const", bufs=1))
        persist = ctx.enter_context(tc.tile_pool(name="persist", bufs=1))
        wts = ctx.enter_context(tc.tile_pool(name="wts", bufs=1))
        attn_pool = ctx.enter_context(tc.tile_pool(name="attn", bufs=1))
        outp = ctx.enter_context(tc.tile_pool(name="outp", bufs=1))

        # ---- on-chip consts (memsets + GpSimd affine_select; no DMAs) ----
        shift_b = const.tile([128, 1], f32)
        nc.vector.memset(shift_b, -EXP_SHIFT)
        wf = const.tile([128, 512], f32)
        nc.vector.memset(wf, 1.0)
        wh4 = const.tile([128, 512], DT)
        nc.vector.tensor_copy(wh4, wf)
        zf = const.tile([SEQ, N_REP * 128], f32)
        nc.vector.memset(zf, 0.0)
        # ident[p, c] = (p == c)
        ident = const.tile([128, 128], DT)
        nc.gpsimd.affine_select(ident, wh4[:, 0:128], pattern=[[-1, 128]],
                                compare_op=ALU.is_equal, fill=0.0,
                                base=0, channel_multiplier=1)
        # sel4[k, h*128+a] = (k == h): stationary selector for recip broadcast
        sel4 = const.tile([N_REP, N_REP * 128], DT)
        nc.gpsimd.affine_select(sel4, wh4[0:N_REP, 0:N_REP * 128],
                                pattern=[[-1, N_REP], [0, 128]],
                                compare_op=ALU.is_equal, fill=0.0,
                                base=0, channel_multiplier=1)
        # mneg4[s, h, c] = 0 where c <= s else MASK_NEG (causal mask, f32)
        mneg4 = const.tile([SEQ, N_REP, 128], f32)
        nc.gpsimd.affine_select(mneg4.rearrange("p a b -> p (a b)"), zf,
                                pattern=[[0, N_REP], [-1, 128]],
                                compare_op=ALU.is_ge, fill=MASK_NEG,
                                base=0, channel_multiplier=1)

        # ---- input stream (sync queue, consumption order) ----
        # order: x, kcT, vct, wq0..3, wk, wv, wo0..6, wol(2 halves)
        xt = persist.tile([128, NK, SEQ], DT)
        nc.sync.dma_start(out=xt.rearrange("p a b -> p (a b)"), in_=xT_d[:, :])
        KT = persist.tile([128, P + SEQ], DT)
        nc.sync.dma_start(out=KT[:, 0:P], in_=kcT_d[:, :])
        vsb = persist.tile([128, NOLD, HEAD_DIM], DT)
        nc.sync.dma_start(out=vsb.rearrange("p a b -> p (a b)"), in_=vc_d[:, :])
        H = NK * HEAD_DIM
        wq_tiles = []
        for h in range(N_REP):
            t_ = wts.tile([128, NK, HEAD_DIM], DT, tag="wq", bufs=4, name=f"wq{h}")
            tv = t_.rearrange("p a b -> p (a b)")
            if h == 0:
                nc.sync.dma_start(out=tv[:, 0:H // 2], in_=wq_d[h, :, 0:H // 2])
                nc.sync.dma_start(out=tv[:, H // 2:], in_=wq_d[h, :, H // 2:])
            else:
                nc.sync.dma_start(out=tv, in_=wq_d[h, :, :])
            wq_tiles.append(t_)
        wkt = wts.tile([128, NK, HEAD_DIM], DT, tag="wk", bufs=1)
        kv_ = wkt.rearrange("p a b -> p (a b)")
        nc.sync.dma_start(out=kv_[:, 0:H // 2], in_=wk_d[:, 0:H // 2])
        nc.sync.dma_start(out=kv_[:, H // 2:], in_=wk_d[:, H // 2:])
        wvt = wts.tile([128, NK, HEAD_DIM], DT, tag="wv", bufs=1)
        vv_ = wvt.rearrange("p a b -> p (a b)")
        nc.sync.dma_start(out=vv_[:, 0:H // 2], in_=wv_d[:, 0:H // 2])
        nc.sync.dma_start(out=vv_[:, H // 2:], in_=wv_d[:, H // 2:])
        # last wo chunk is half-major on the host ([2, N_REP, 256]) and split
        # into two DMAs so the final out pipeline covers 256 cols, not 512
        wo_tiles = []
        NWO = DIM // 512
        for n in range(NWO - 1):
            t_ = wts.tile([128, N_REP, 512], DT, tag="wo", bufs=8, name=f"wo{n}")
            nc.sync.dma_start(out=t_.rearrange("p a b -> p (a b)"), in_=wo_d[n, :, :])
            wo_tiles.append(t_)
        wol = wts.tile([128, 2, N_REP, 256], DT, tag="wo", bufs=8, name="wol")
        wolv = wol.rearrange("p s a b -> p (s a b)")
        nc.sync.dma_start(out=wolv[:, 0:N_REP * 256], in_=wo_d[NWO - 1, :, 0:N_REP * 256])
        nc.sync.dma_start(out=wolv[:, N_REP * 256:], in_=wo_d[NWO - 1, :, N_REP * 256:])

        # ---- persistent SBUF state ----
        qT_sb = persist.tile([128, N_REP, SEQ], DT)
        vnew_sb = persist.tile([SEQ, HEAD_DIM], DT)
        attn_new = persist.tile([SEQ, N_REP, 128], DT)
        attnT = persist.tile([128, NCH, N_REP, 128], DT)
        yT_sb = persist.tile([128, N_REP, SEQ], DT)
        zp = persist.tile([128, N_REP, NB2 + 1], f32)
        zsum = persist.tile([128, N_REP, 1], f32)
        recip4 = persist.tile([128, N_REP], f32)
        recip4h = persist.tile([128, N_REP], DT)
        r4T_sb = persist.tile([N_REP, 128], DT)
        rb_sb = persist.tile([128, N_REP, SEQ], DT)

        with tc.tile_pool(name="ps", bufs=1, space="PSUM") as ps:
            qT_ps = ps.tile([128, N_REP, SEQ], f32, tag="qT")
            yT_ps = ps.tile([128, N_REP * SEQ], f32, tag="yT")

            # PE warmup during the x/kcT/vct/wq0 stream: flips the HAM clock
            # gate to full rate before the q projection starts. Operands are
            # memset on-chip so the warmups have no DMA dependency.
            warm = ps.tile([128, 4, 128], f32, tag="tp", bufs=2)
            wv_ = warm.rearrange("p a b -> p (a b)")
            for _ in range(N_WARM):
                nc.tensor.matmul(wv_, wh4[:, 0:128], wh4[:],
                                 start=True, stop=True)

            attn_t = {}

            def q_head(h):
                for j in range(NK):
                    nc.tensor.matmul(qT_ps[:, h, :], wq_tiles[h][:, j, :],
                                     xt[:, j, :], start=(j == 0), stop=(j == NK - 1))
                nc.vector.tensor_copy(qT_sb[:, h, :], qT_ps[:, h, :])

            def s(h, b2):
                # 1024-wide exp blocks (2 score matmuls each), double-buffered
                # so block n+1's score matmuls run under block n's exp: the
                # chain is ACT-rate, not a per-block PSUM round-trip
                sc = ps.tile([SEQ, 1024], f32, tag="sc", bufs=2, name=f"sc{h}{b2}")
                for i in range(2):
                    off = b2 * 1024 + i * SB
                    nc.tensor.matmul(sc[:, i * SB:(i + 1) * SB], qT_sb[:, h, :],
                                     KT[:, off:off + SB], start=True, stop=True)
                a = attn_t[h][:, b2 * 1024:(b2 + 1) * 1024]
                nc.scalar.activation(a, sc[:], AFT.Exp, scale=SCALE,
                                     bias=shift_b[:],
                                     accum_out=zp[:, h, b2:b2 + 1])

            def t(h, b, copy_eng="v"):
                # transpose attn[h] chunks 4b..4b+3 -> attnT[:, 4b:4b+4, h, :]
                tp = ps.tile([128, 4, 128], f32, tag="tp", bufs=2, name=f"tp{h}{b}")
                for i in range(4):
                    c = 4 * b + i
                    nc.tensor.matmul(tp[:, i, :], attn_t[h][:, c * 128:(c + 1) * 128],
                                     ident[:], start=True, stop=True)
                dst = attnT[:, 4 * b:4 * b + 4, h, :]
                if copy_eng == "v":
                    nc.vector.tensor_copy(dst, tp[:])
                else:
                    nc.scalar.copy(dst, tp[:])

            def av(b):
                for i in range(4):
                    c = 4 * b + i
                    nc.tensor.matmul(yT_ps[:], vsb[:, c, :], attnT[:, c, :, :],
                                     start=(c == 0), stop=False)

            def newattn(h):
                attn_t[h] = attn_pool.tile([SEQ, P], DT, tag="attn", bufs=2,
                                           name=f"at{h}")

            # ---- pipelined emission (follows the stream order) ----
            # Per head: q(h) FIRST (its dep, the wq DMA, is the earliest),
            # then the previous head's transposes interleaved with the score
            # blocks — everything the PE FIFO reaches has its deps met, so
            # no head-of-line stalls.
            q_head(0)
            newattn(0)
            s(0, 0); s(0, 1)
            q_head(1)
            newattn(1)
            s(1, 0); s(1, 1)
            t(0, 0); t(0, 1); t(0, 2); t(0, 3)
            q_head(2)
            newattn(2)
            s(2, 0); s(2, 1)
            t(1, 0); t(1, 1); t(1, 2); t(1, 3)
            q_head(3)
            newattn(3)
            s(3, 0); s(3, 1)
            t(2, 0); t(2, 1); t(2, 2); t(2, 3)

            # k projection first: its dep (the wk stream) lands while the
            # head-3 exps are still running, so it must not queue behind the
            # head-3 transposes in the PE FIFO
            kv_ps = ps.tile([128, 4, 128], f32, tag="qT", name="kv")
            for j in range(NK):
                nc.tensor.matmul(kv_ps[:, 0, :], wkt[:, j, :], xt[:, j, :],
                                 start=(j == 0), stop=(j == NK - 1))
            nc.vector.tensor_copy(KT[:, P:P + SEQ], kv_ps[:, 0, :])

            # new-block scores for all heads (masked via mneg4 add pre-exp)
            nsc = ps.tile([128, N_REP, 128], f32, tag="sc", bufs=2, name="nsc")
            for h in range(N_REP):
                nc.tensor.matmul(nsc[:, h, :], qT_sb[:, h, :], KT[:, P:P + SEQ],
                                 start=True, stop=True)
            nscv = nsc.rearrange("p a b -> p (a b)")
            nc.vector.tensor_add(nscv, nscv, mneg4.rearrange("p a b -> p (a b)"))
            nc.scalar.activation(attn_new.rearrange("p a b -> p (a b)"), nscv,
                                 AFT.Exp, scale=SCALE, bias=shift_b[:])

            # v projection (wv is fully streamed by now)
            for j in range(NK):
                nc.tensor.matmul(kv_ps[:, 1, :], xt[:, j, :], wvt[:, j, :],
                                 start=(j == 0), stop=(j == NK - 1))
            nc.scalar.copy(vnew_sb[:], kv_ps[:, 1, :])

            # head-3 transposes: first two copies on DVE (its queue is short
            # until the z chain), last two on ACT right after the new exp
            t(3, 0, "v"); t(3, 1, "v"); t(3, 2, "s"); t(3, 3, "s")

            # AV chunks interleaved right behind the transpose copies
            av(0); av(1)

            # new-block attn transpose
            tpn = ps.tile([128, 4, 128], f32, tag="tp", bufs=2)
            for h in range(N_REP):
                nc.tensor.matmul(tpn[:, h, :], attn_new[:, h, :], ident[:],
                                 start=True, stop=True)
            nc.scalar.copy(attnT[:, NOLD, :, :], tpn[:])

            av(2); av(3)

            # softmax denominator chain on DVE
            nc.vector.reduce_sum(zp[:, :, NB2:NB2 + 1], attn_new[:],
                                 axis=mybir.AxisListType.X)
            nc.vector.reduce_sum(zsum[:], zp[:], axis=mybir.AxisListType.X)
            nc.vector.reciprocal(recip4[:], zsum.rearrange("p a b -> p (a b)"))
            nc.vector.tensor_copy(recip4h[:], recip4[:])

            r4T_ps = ps.tile([N_REP, 128], f32, tag="tp", bufs=2, name="r4T")
            nc.tensor.matmul(r4T_ps[:], recip4h[:], ident[:], start=True, stop=True)
            nc.scalar.copy(r4T_sb[:], r4T_ps[:])
            rb_ps = ps.tile([128, N_REP, SEQ], f32, tag="sc", bufs=2, name="rb")
            for h in range(N_REP):
                nc.tensor.matmul(rb_ps[:, h, :], sel4[:, h * 128:(h + 1) * 128],
                                 r4T_sb[:], start=True, stop=True)

            nc.scalar.copy(rb_sb[:], rb_ps[:])
            nc.tensor.matmul(yT_ps[:], vnew_sb[:], attnT[:, NOLD, :, :],
                             start=False, stop=True)
            nc.vector.tensor_mul(yT_sb.rearrange("p h s -> p (h s)"), yT_ps[:],
                                 rb_sb.rearrange("p h s -> p (h s)"))

            # ---- wo (row-parallel partial), chasing the wo stream ----
            # ob copies on DVE, out-DMA issue on the otherwise-idle ACT queue
            # ob copies alternate DVE/ACT so consecutive po chunks' copies
            # overlap (the tp-slot rotation waits on the copy two chunks back)
            for n in range(NWO - 1):
                po = ps.tile([SEQ, SB], f32, tag="tp" if n % 2 == 0 else "sc",
                             bufs=2, name=f"po{n}")
                for h in range(N_REP):
                    nc.tensor.matmul(po[:], yT_sb[:, h, :], wo_tiles[n][:, h, :],
                                     start=(h == 0), stop=(h == N_REP - 1))
                ob = outp.tile([SEQ, 512], DT, tag="ob", bufs=3, name=f"ob{n}")
                if n % 2 == 0:
                    nc.vector.tensor_copy(ob[:], po[:])
                else:
                    nc.scalar.copy(ob[:], po[:])
                nc.sync.dma_start(out=out_d[:, n * 512:(n + 1) * 512], in_=ob[:])
            for half in range(2):
                po = ps.tile([SEQ, SB], f32, tag="tp" if half == 0 else "sc",
                             bufs=2, name=f"pol{half}")
                for h in range(N_REP):
                    nc.tensor.matmul(po[:, 0:256], yT_sb[:, h, :],
                                     wol[:, half, h, :],
                                     start=(h == 0), stop=(h == N_REP - 1))
                ob = outp.tile([SEQ, 512], DT, tag="ob", bufs=3, name=f"obl{half}")
                if half == 0:
                    nc.vector.tensor_copy(ob[:, 0:256], po[:, 0:256])
                else:
                    nc.scalar.copy(ob[:, 0:256], po[:, 0:256])
                off = (NWO - 1) * 512 + half * 256
                nc.sync.dma_start(out=out_d[:, off:off + 256], in_=ob[:, 0:256])

    nc.finalize()
    return nc


def _get_nc(P):
    if P not in _nc_cache:
        _nc_cache[P] = _build_nc(P)
    return _nc_cache[P]


def prep_in_maps(x, input_pos, k_cache, v_cache, wq, wk, wv, wo):
    P = int(input_pos)
    NOLD = P // 128
    x2 = np.asarray(x, dtype=np.float32).reshape(SEQ, DIM)
    k_cache = np.asarray(k_cache, dtype=np.float32)
    v_cache = np.asarray(v_cache, dtype=np.float32)
    wq = np.asarray(wq, dtype=np.float32)
    wk = np.asarray(wk, dtype=np.float32)
    wv = np.asarray(wv, dtype=np.float32)
    wo = np.asarray(wo, dtype=np.float32)

    xT = x2.T.astype(np.float16)                              # [DIM, SEQ]
    xTt = np.ascontiguousarray(
        xT.reshape(NK, 128, SEQ).transpose(1, 0, 2).reshape(128, NK * SEQ))

    def chunkT(wT):  # [DIM, C] -> [128, NK*C]: out[p, j*C+c] = wT[j*128+p, c]
        C = wT.shape[1]
        return np.ascontiguousarray(
            wT.reshape(NK, 128, C).transpose(1, 0, 2).reshape(128, NK * C))

    in_maps = []
    for c in range(N_CORES):
        wq_c = wq[c * 512:(c + 1) * 512]                      # [512, DIM]
        wqt = np.stack([chunkT(wq_c[h * 128:(h + 1) * 128].T.astype(np.float16))
                        for h in range(N_REP)])               # [4, 128, NK*128]
        wkt = chunkT(wk[c * 128:(c + 1) * 128].T.astype(np.float16))
        wvt = chunkT(wv[c * 128:(c + 1) * 128].T.astype(np.float16))
        woT = wo[:, c * 512:(c + 1) * 512].T.astype(np.float16)  # [512, DIM]
        # wot[n][p, h*512+col] = woT[h*128+p, n*512+col]
        wot = np.ascontiguousarray(
            woT.reshape(N_REP, 128, DIM // 512, 512)
            .transpose(2, 1, 0, 3).reshape(DIM // 512, 128, N_REP * 512))
        # last chunk half-major: wot[-1][p, s*1024 + h*256 + col]
        wol = wot[-1].reshape(128, N_REP, 2, 256).transpose(0, 2, 1, 3)
        wot = np.concatenate(
            [wot[:-1], wol.reshape(1, 128, N_REP * 512)], axis=0)
        m = {
            "xTt": xTt,
            "wqt": wqt,
            "wkt": wkt,
            "wvt": wvt,
            "wot": wot,
            "kcT": np.ascontiguousarray(k_cache[0, c, :P].T.astype(np.float16)),
            "vct": np.ascontiguousarray(
                v_cache[0, c, :P].astype(np.float16)
                .reshape(NOLD, 128, HEAD_DIM).transpose(1, 0, 2).reshape(128, -1)),
        }
        in_maps.append(m)
    return P, in_maps


def kernel(x, input_pos, k_cache, v_cache, wq, wk, wv, wo):
    from concourse.bass_utils import run_bass_kernel_spmd

    P, in_maps = prep_in_maps(x, input_pos, k_cache, v_cache, wq, wk, wv, wo)
    nc = _get_nc(P)
    res = run_bass_kernel_spmd(nc, in_maps, core_ids=list(range(N_CORES)))
    out = np.zeros((SEQ, DIM), dtype=np.float32)
    for r in res.results:
        out += r["out"].astype(np.float32)
    return out.reshape(1, SEQ, DIM)


if __name__ == "__main__":
    rng = np.random.default_rng(0)
    ins = {
        "x": rng.standard_normal((1, SEQ, DIM), dtype=np.float32),
        "input_pos": 2048,
        "k_cache": rng.standard_normal((1, N_KV_HEADS, MAX_SEQ, HEAD_DIM),
                                       dtype=np.float32),
        "v_cache": rng.standard_normal((1, N_KV_HEADS, MAX_SEQ, HEAD_DIM),
                                       dtype=np.float32),
        "wq": rng.standard_normal((N_HEADS * HEAD_DIM, DIM), dtype=np.float32) * 0.02,
        "wk": rng.standard_normal((N_KV_HEADS * HEAD_DIM, DIM),
                                  dtype=np.float32) * 0.02,
        "wv": rng.standard_normal((N_KV_HEADS * HEAD_DIM, DIM),
                                  dtype=np.float32) * 0.02,
        "wo": rng.standard_normal((DIM, N_HEADS * HEAD_DIM), dtype=np.float32) * 0.02,
    }
    out = kernel(**ins)
    print("out", out.shape, out.dtype, float(np.abs(out).max()))
